# revision 2
# baseline (speedup 1.0000x reference)
"""GTCN kernel on 8 Trainium2 NeuronCores.

Strategy (per sharding_hint): data-parallel over batch B=64 across the 8
cores (8 samples each); all params replicated. The whole network runs as a
single XLA program per core via jax.jit + shard_map on the axon-tunneled
NeuronCores. Exact train-mode BatchNorm is kept by all-reducing the
per-timestep (and per-channel, for the temporal convs) sum/sum-of-squares
statistics across cores with lax.psum before normalizing.

Shapes hardcoded per spec: B=64, C=3, T=512, V=25, Hd=64, E=48, NC=60.
The edge scatter-add is folded into a dense (V,V) count-adjacency matmul
(duplicate edges accumulate, matching the reference's .at[].add).
"""

import numpy as np

BN_EPS = 1e-5
_B, _C, _T, _V, _HD, _NC = 64, 3, 512, 25, 64, 60
_NCORES = 8

_cache = {}


def _build():
    if "fn" in _cache:
        return _cache["fn"]

    import jax
    import jax.numpy as jnp
    from jax import lax
    from jax.sharding import Mesh, PartitionSpec as P

    try:
        from jax.experimental.shard_map import shard_map
    except ImportError:  # newer jax
        from jax.experimental import shard_map as _sm
        shard_map = _sm.shard_map

    try:
        devs = jax.devices("axon")
    except RuntimeError:
        devs = jax.devices()
    devs = devs[:_NCORES]
    mesh = Mesh(np.asarray(devs), ("b",))

    def _gcn(Xt, A, W, sW, sb, gamma, beta):
        # Xt: (b_local, T, V, Fin)
        H = Xt @ sW + sb + jnp.einsum("ds,btsf->btdf", A, Xt @ W)
        s = lax.psum(H.sum(axis=(0, 2)), "b")          # (T, Hd)
        sq = lax.psum((H * H).sum(axis=(0, 2)), "b")   # (T, Hd)
        denom = float(_B * _V)
        mean = s / denom
        var = sq / denom - mean * mean
        scale = gamma * lax.rsqrt(var + BN_EPS)        # (T, Hd)
        shift = beta - mean * scale
        return jax.nn.relu(H * scale[None, :, None, :] + shift[None, :, None, :])

    def _conv_bn(x, w, b, gamma, beta, dilation, padding):
        # x: (b_local, Cin, T), w: (O, Cin, 3)
        y = lax.conv_general_dilated(
            x, w, window_strides=(1,), padding=[(padding, padding)],
            rhs_dilation=(dilation,),
            dimension_numbers=("NCH", "OIH", "NCH"))
        y = y + b[None, :, None]
        s = lax.psum(y.sum(axis=(0, 2)), "b")          # (O,)
        sq = lax.psum((y * y).sum(axis=(0, 2)), "b")
        denom = float(_B * _T)
        mean = s / denom
        var = sq / denom - mean * mean
        scale = gamma * lax.rsqrt(var + BN_EPS)
        shift = beta - mean * scale
        return jax.nn.relu(y * scale[None, :, None] + shift[None, :, None])

    def _fwd(X, A, W1, s1W, s1b, g1, b1, W2, s2W, s2b, g2, b2,
             c1W, c1b, tg1, tb1, c2W, c2b, tg2, tb2, fcW, fcb):
        # X: (b_local, C, T, V)
        Xt = jnp.transpose(X, (0, 2, 3, 1))            # (b, T, V, C)
        H = _gcn(Xt, A, W1, s1W, s1b, g1, b1)
        H = _gcn(H, A, W2, s2W, s2b, g2, b2)
        z = jnp.transpose(H, (0, 2, 3, 1)).reshape(H.shape[0], _V * _HD, _T)
        z = _conv_bn(z, c1W, c1b, tg1, tb1, dilation=1, padding=1)
        z = _conv_bn(z, c2W, c2b, tg2, tb2, dilation=2, padding=2)
        z = z.mean(axis=2)                             # (b, 128)
        return z @ fcW + fcb                           # (b, NC)

    n_rep = 21  # number of replicated operands after X
    fn = jax.jit(shard_map(
        _fwd, mesh=mesh,
        in_specs=(P("b"),) + (P(),) * n_rep,
        out_specs=P("b"),
        check_rep=False,
    ))
    _cache["fn"] = fn
    return fn


def kernel(X, edge_index, W1, s1W, s1b, g1, b1, W2, s2W, s2b, g2, b2,
           c1W, c1b, tg1, tb1, c2W, c2b, tg2, tb2, fcW, fcb):
    X = np.ascontiguousarray(np.asarray(X, dtype=np.float32))
    edge_index = np.asarray(edge_index)

    # Edge list -> count adjacency (duplicates accumulate, matching the
    # reference's scatter-add over dst of per-edge messages).
    A = np.zeros((_V, _V), dtype=np.float32)
    np.add.at(A, (edge_index[:, 1], edge_index[:, 0]), 1.0)

    args = [np.asarray(a, dtype=np.float32) for a in (
        A, W1, s1W, s1b, g1, b1, W2, s2W, s2b, g2, b2,
        c1W, c1b, tg1, tb1, c2W, c2b, tg2, tb2, fcW, fcb)]

    fn = _build()
    out = fn(X, *args)
    return np.asarray(out, dtype=np.float32)
